# revision 7
# baseline (speedup 1.0000x reference)
"""Trainium2 Bass kernel for the GRU greedy decoder (nn_Decoder).

Strategy (8 NeuronCores):
  - W_out is vocab-sharded: each core keeps a [1024, 4000] slice of W_out.T
    resident in SBUF and computes logits for its 4000-vocab shard.
  - The GRU is hidden-sharded: core c computes gates for hidden units
    [128c, 128c+128) and the per-step hidden state is AllGathered (transposed,
    so it lands ready to use as the matmul stationary operand).
  - Greedy argmax + log_softmax stats (max / sumexp / argmax index) are
    reduced locally per core, exchanged with a tiny AllGather each step, and
    combined identically on every core.
  - The embedding lookup is an indirect DMA row gather using the token ids.
All compute is fp32 end to end (argmax of the token chain requires it).
"""

import sys

for _p in ("/opt/trn_rl_repo", "/root/.axon_site/_ro/trn_rl_repo"):
    if _p not in sys.path:
        sys.path.insert(0, _p)

import os
import numpy as np

import concourse.bass as bass
import concourse.bacc as bacc
import concourse.tile as tile
import concourse.mybir as mybir
from concourse import bass_utils
from concourse.bass import IndirectOffsetOnAxis

F32 = mybir.dt.float32
U32 = mybir.dt.uint32
U8 = mybir.dt.uint8
AF = mybir.ActivationFunctionType
ALU = mybir.AluOpType

V, H, B = 32000, 1024, 64
T = int(os.environ.get("DEC_T", "64"))  # sequence length (64 for the real problem)
NCORE = 8
VS = V // NCORE          # 4000 vocab per core
HS = H // NCORE          # 128 hidden units per core
GS = 3 * HS              # 384 gate outputs per core
KC = H // 128            # 8 contraction chunks
VCH = 500                # vocab chunk per PSUM bank
HALF = 2000              # vocab per partition-half (col-tiled projection)
SOS = 1

_CACHE = {}


def _build():
    """Builds and compiles the SPMD Bass program (identical on all cores)."""
    nc = bacc.Bacc("TRN2", target_bir_lowering=False, debug=False, num_devices=NCORE)

    # ---- kernel I/O -------------------------------------------------------
    din = {}
    def inp(name, shape, dtype=F32):
        din[name] = nc.dram_tensor(name, list(shape), dtype, kind="ExternalInput")
        return din[name]

    emb_t = inp("emb", [V, H])
    wot_t = inp("wot", [128, KC, VS])       # W_out.T shard: [p, k, v]
    wih_t = inp("wih", [128, KC, GS])       # W_ih.T gate-slice: [p, k, g]
    whh_t = inp("whh", [128, KC, GS])
    bo_t = inp("bo", [1, VS])
    brz_t = inp("brz", [1, 2 * HS])         # b_ih+b_hh for r,z rows
    bin_t = inp("bin", [1, HS])             # b_ih for n rows
    bhn_t = inp("bhn", [1, HS])             # b_hh for n rows
    h0sl_t = inp("h0sl", [B, HS])
    h0T_t = inp("h0T", [128, KC, B])
    x0T_t = inp("x0T", [128, KC, B])
    ident_t = inp("ident", [128, 128])
    voff_t = inp("voff", [128, 1])          # per-core vocab offset (c*VS)

    out_logp = nc.dram_tensor("out_logp", [B, T, VS], F32, kind="ExternalOutput")
    out_hsl = nc.dram_tensor("out_hsl", [B, HS], F32, kind="ExternalOutput")

    RG = [list(range(NCORE))]

    with tile.TileContext(nc) as tc:
        with tc.tile_pool(name="wp", bufs=1) as wp, \
             tc.tile_pool(name="sp", bufs=2) as sp, \
             tc.tile_pool(name="pp", bufs=1, space="PSUM") as pp, \
             tc.tile_pool(name="dp", bufs=2, space="DRAM") as dp:

            # ---- resident weights / constants -----------------------------
            wot = wp.tile([128, KC, VS], F32, name="wot_sb")
            nc.sync.dma_start(wot[:], wot_t[:, :, :])
            wih = wp.tile([128, KC, GS], F32, name="wih_sb")
            nc.sync.dma_start(wih[:], wih_t[:, :, :])
            whh = wp.tile([128, KC, GS], F32, name="whh_sb")
            nc.sync.dma_start(whh[:], whh_t[:, :, :])
            bo = wp.tile([1, VS], F32, name="bo_sb")
            nc.sync.dma_start(bo[:], bo_t[:, :])
            brz = wp.tile([1, 2 * HS], F32, name="brz_sb")
            nc.sync.dma_start(brz[:], brz_t[:, :])
            bin_ = wp.tile([1, HS], F32, name="bin_sb")
            nc.sync.dma_start(bin_[:], bin_t[:, :])
            bhn = wp.tile([1, HS], F32, name="bhn_sb")
            nc.sync.dma_start(bhn[:], bhn_t[:, :])
            ident = wp.tile([128, 128], F32, name="ident_sb")
            nc.sync.dma_start(ident[:], ident_t[:, :])
            voff = wp.tile([128, 1], F32, name="voff_sb")
            nc.sync.dma_start(voff[:], voff_t[:, :])
            x0T = wp.tile([128, KC, B], F32, name="x0T_sb")
            nc.sync.dma_start(x0T[:], x0T_t[:, :, :])
            ones = wp.tile([1, B], F32, name="ones_sb")
            nc.vector.memset(ones[:], 1.0)
            bigc = wp.tile([128, 2 * NCORE], F32, name="bigc_sb")
            nc.vector.memset(bigc[:], 1.0e9)

            h_sl = wp.tile([B, HS], F32, name="h_sl0")
            nc.sync.dma_start(h_sl[:], h0sl_t[:, :])
            hT = wp.tile([128, KC, B], F32, name="hT0")
            nc.sync.dma_start(hT[:], h0T_t[:, :, :])

            prev = None  # (logits_sb, lse_bc) of the previous step

            for t in range(T):
                # ---- write back step t-1 log-probs (off critical path) ----
                if prev is not None:
                    p_lg, p_lse = prev
                    outb = sp.tile([128, HALF], F32, tag="outb", bufs=1,
                                   name=f"outb{t}")
                    nc.gpsimd.tensor_scalar(outb[:], p_lg[:], p_lse[:], None,
                                            op0=ALU.subtract)
                    nc.sync.dma_start(
                        out_logp[:, t - 1:t, 0:HALF],
                        outb[0:B, :].rearrange("b (x v) -> b x v", x=1))
                    nc.sync.dma_start(
                        out_logp[:, t - 1:t, HALF:VS],
                        outb[B:128, :].rearrange("b (x v) -> b x v", x=1))

                # ---- GRU: gates psum (bias + gh + gi) ---------------------
                rz_ps = pp.tile([B, 2 * HS], F32, tag="rz", name=f"rz{t}")
                gin_ps = pp.tile([B, HS], F32, tag="gin", name=f"gin{t}")
                ghn_ps = pp.tile([B, HS], F32, tag="ghn", name=f"ghn{t}")
                nc.tensor.matmul(rz_ps[:], ones[:], brz[:], start=True, stop=False)
                nc.tensor.matmul(gin_ps[:], ones[:], bin_[:], start=True, stop=False)
                nc.tensor.matmul(ghn_ps[:], ones[:], bhn[:], start=True, stop=False)
                for k in range(KC):
                    nc.tensor.matmul(rz_ps[:], hT[:, k, :], whh[:, k, 0:2 * HS],
                                     start=False, stop=False)
                    nc.tensor.matmul(ghn_ps[:], hT[:, k, :], whh[:, k, 2 * HS:GS],
                                     start=False, stop=(k == KC - 1))

                # gi: x comes from the gather (t>0) or the precomputed x0T (t=0)
                if t == 0:
                    for k in range(KC):
                        nc.tensor.matmul(rz_ps[:], x0T[:, k, :], wih[:, k, 0:2 * HS],
                                         start=False, stop=(k == KC - 1))
                        nc.tensor.matmul(gin_ps[:], x0T[:, k, :], wih[:, k, 2 * HS:GS],
                                         start=False, stop=(k == KC - 1))
                else:
                    for k in range(KC):
                        xp_ps = pp.tile([128, B], F32, tag="xp", bufs=2,
                                        name=f"xp{t}_{k}")
                        nc.tensor.transpose(xp_ps[:], x_sb[:, 128 * k:128 * (k + 1)],
                                            ident[0:B, 0:B])
                        xt_sb = sp.tile([128, B], F32, tag="xt", bufs=2,
                                        name=f"xt{t}_{k}")
                        nc.vector.tensor_copy(xt_sb[:], xp_ps[:])
                        nc.tensor.matmul(rz_ps[:], xt_sb[:], wih[:, k, 0:2 * HS],
                                         start=False, stop=(k == KC - 1))
                        nc.tensor.matmul(gin_ps[:], xt_sb[:], wih[:, k, 2 * HS:GS],
                                         start=False, stop=(k == KC - 1))

                # ---- gate nonlinearities ---------------------------------
                rz_sb = sp.tile([B, 2 * HS], F32, tag="rzsb", name=f"rzsb{t}")
                nc.scalar.activation(rz_sb[:], rz_ps[:], AF.Sigmoid)
                t1 = sp.tile([B, HS], F32, tag="t1", name=f"t1_{t}")
                nc.vector.tensor_tensor(t1[:], rz_sb[:, 0:HS], ghn_ps[:], ALU.mult)
                t2 = sp.tile([B, HS], F32, tag="t2", name=f"t2_{t}")
                nc.vector.tensor_tensor(t2[:], t1[:], gin_ps[:], ALU.add)
                n_sb = sp.tile([B, HS], F32, tag="nsb", name=f"nsb{t}")
                nc.scalar.activation(n_sb[:], t2[:], AF.Tanh)
                d_sb = sp.tile([B, HS], F32, tag="dsb", name=f"dsb{t}")
                nc.vector.tensor_tensor(d_sb[:], h_sl[:], n_sb[:], ALU.subtract)
                zd_sb = sp.tile([B, HS], F32, tag="zdsb", name=f"zdsb{t}")
                nc.vector.tensor_tensor(zd_sb[:], rz_sb[:, HS:2 * HS], d_sb[:],
                                        ALU.mult)
                h_new = sp.tile([B, HS], F32, tag="hsl", name=f"hsl{t}")
                nc.vector.tensor_tensor(h_new[:], n_sb[:], zd_sb[:], ALU.add)
                h_sl = h_new

                # ---- AllGather the new hidden state (transposed) ----------
                hxp_ps = pp.tile([128, B], F32, tag="xp", bufs=2, name=f"hxp{t}")
                nc.tensor.transpose(hxp_ps[:], h_sl[:], ident[0:B, 0:B])
                hTo = sp.tile([128, B], F32, tag="hTo", name=f"hTo{t}")
                nc.vector.tensor_copy(hTo[:], hxp_ps[:])
                ch_in = dp.tile([128, B], F32, tag="chin", name=f"chin{t}")
                nc.sync.dma_start(ch_in[:], hTo[:])
                ch_out = dp.tile([KC * 128, B], F32, tag="chout",
                                 addr_space="Shared", name=f"chout{t}")
                nc.gpsimd.collective_compute(
                    "AllGather", ALU.bypass, replica_groups=RG,
                    ins=[ch_in[:]], outs=[ch_out[:]])
                hT_new = sp.tile([128, KC, B], F32, tag="hT", name=f"hT{t}")
                for k in range(KC):
                    nc.sync.dma_start(hT_new[:, k, :],
                                      ch_out[128 * k:128 * (k + 1), :])
                hT = hT_new

                # ---- projection: logits = h_new @ W_out_shard.T + b -------
                # col-tiled: partitions 0:64 do vocab [0,2000), 64:128 do
                # [2000,4000); PE col-groups run the two streams concurrently.
                lg = sp.tile([128, HALF], F32, tag="lg", bufs=1, name=f"lg{t}")
                pj_tiles = []
                for p in range(2):
                    js = (2 * p, 2 * p + 1)
                    tiles = {}
                    for j in js:
                        pj = pp.tile([128, VCH], F32, tag=f"pj{j % 2}",
                                     name=f"pj{t}_{j}")
                        tiles[j] = pj
                        nc.tensor.matmul(pj[0:B, :], ones[:],
                                         bo[:, VCH * j:VCH * (j + 1)],
                                         start=True, stop=False,
                                         tile_position=(0, 0))
                        nc.tensor.matmul(pj[B:128, :], ones[:],
                                         bo[:, HALF + VCH * j:HALF + VCH * (j + 1)],
                                         start=True, stop=False,
                                         tile_position=(0, 64))
                    for k in range(KC):
                        last = k == KC - 1
                        for j in js:
                            pj = tiles[j]
                            nc.tensor.matmul(
                                pj[0:B, :], hT[:, k, :],
                                wot[:, k, VCH * j:VCH * (j + 1)],
                                start=False, stop=last, tile_position=(0, 0))
                            nc.tensor.matmul(
                                pj[B:128, :], hT[:, k, :],
                                wot[:, k, HALF + VCH * j:HALF + VCH * (j + 1)],
                                start=False, stop=last, tile_position=(0, 64))
                    for j in js:
                        nc.vector.tensor_copy(lg[:, VCH * j:VCH * (j + 1)],
                                              tiles[j][:])

                # ---- local softmax/argmax stats (per partition-half) ------
                # Each partition's 2000-wide row is one (batch, vocab-half)
                # pair; stats are exchanged per-half and combined from the
                # AllGathered [8 cores x 2 halves] table, so no op ever
                # crosses partition bases.
                mx8 = sp.tile([128, 8], F32, tag="mx8", name=f"mx8{t}")
                nc.vector.max(mx8[:], lg[:])
                if t < T - 1:
                    midx = sp.tile([128, 8], U32, tag="midx", name=f"midx{t}")
                    nc.vector.max_index(midx[:], mx8[:], lg[:])
                neg_m = sp.tile([128, 1], F32, tag="negm", name=f"negm{t}")
                nc.vector.tensor_scalar_mul(neg_m[:], mx8[:, 0:1], -1.0)
                stats = sp.tile([128, 4], F32, tag="stats", name=f"stats{t}")
                s4 = sp.tile([128, 4], F32, tag="s4", name=f"s4_{t}")
                for j in range(4):
                    sink = pp.tile([128, VCH], F32, tag=f"pj{j % 2}",
                                   name=f"esink{t}_{j}")
                    nc.scalar.activation(sink[:], lg[:, VCH * j:VCH * (j + 1)],
                                         AF.Exp, bias=neg_m[:],
                                         accum_out=s4[:, j:j + 1])
                # col0: half max, col1: half sumexp, col2: global argmax idx
                nc.vector.tensor_copy(stats[:, 0:1], mx8[:, 0:1])
                nc.vector.reduce_sum(stats[:, 1:2], s4[:],
                                     axis=mybir.AxisListType.X)
                if t < T - 1:
                    idxf = sp.tile([128, 1], F32, tag="idxf", name=f"idxf{t}")
                    nc.vector.tensor_copy(idxf[:], midx[:, 0:1])
                    nc.vector.tensor_scalar_add(idxf[B:128, :], idxf[B:128, :],
                                                float(HALF))
                    nc.vector.tensor_tensor(stats[:, 2:3], idxf[:], voff[:],
                                            ALU.add)

                # ---- exchange stats ---------------------------------------
                cs_in = dp.tile([128, 4], F32, tag="csin", name=f"csin{t}")
                nc.sync.dma_start(cs_in[:], stats[:])
                cs_out = dp.tile([NCORE * 128, 4], F32, tag="csout",
                                 addr_space="Shared", name=f"csout{t}")
                nc.gpsimd.collective_compute(
                    "AllGather", ALU.bypass, replica_groups=RG,
                    ins=[cs_in[:]], outs=[cs_out[:]])
                # Mirror the 16 (core, half) entries into both partition
                # halves so lse/tok come out as plain [128,1] columns.
                allst = sp.tile([128, 2 * NCORE, 4], F32, tag="allst",
                                name=f"allst{t}")
                rd = cs_out[:].rearrange("(c h b) f -> b (c h) f", c=NCORE, h=2)
                nc.sync.dma_start(allst[0:B, :, :], rd)
                nc.sync.dma_start(allst[B:128, :, :], rd)

                # ---- global combine (identical on every core) -------------
                NE = 2 * NCORE
                Mt = sp.tile([128, 1], F32, tag="Mt", name=f"Mt{t}")
                nc.vector.tensor_reduce(Mt[:], allst[:, :, 0:1],
                                        axis=mybir.AxisListType.XY, op=ALU.max)
                dd16 = sp.tile([128, NE], F32, tag="dd16", name=f"dd16{t}")
                nc.vector.tensor_scalar(
                    dd16[:].rearrange("b (c f) -> b c f", f=1),
                    allst[:, :, 0:1], Mt[:], None, op0=ALU.subtract)
                ee16 = sp.tile([128, NE], F32, tag="ee16", name=f"ee16{t}")
                nc.scalar.activation(ee16[:], dd16[:], AF.Exp)
                tt16 = sp.tile([128, NE], F32, tag="tt16", name=f"tt16{t}")
                nc.vector.tensor_tensor(tt16[:].rearrange("b (c f) -> b c f", f=1),
                                        ee16[:].rearrange("b (c f) -> b c f", f=1),
                                        allst[:, :, 1:2], ALU.mult)
                St = sp.tile([128, 1], F32, tag="St", name=f"St{t}")
                nc.vector.reduce_sum(St[:], tt16[:], axis=mybir.AxisListType.X)
                lnS = sp.tile([128, 1], F32, tag="lnS", name=f"lnS{t}")
                nc.scalar.activation(lnS[:], St[:], AF.Ln)
                lse_bc = sp.tile([128, 1], F32, tag="lsebc", name=f"lsebc{t}")
                nc.vector.tensor_tensor(lse_bc[:], Mt[:], lnS[:], ALU.add)

                if t < T - 1:
                    mask16 = sp.tile([128, NE], U8, tag="mask16",
                                     name=f"mask16{t}")
                    nc.vector.tensor_scalar(
                        mask16[:].rearrange("b (c f) -> b c f", f=1),
                        allst[:, :, 0:1], Mt[:], None, op0=ALU.is_equal)
                    c16 = sp.tile([128, NE], F32, tag="c16", name=f"c16{t}")
                    nc.vector.select(c16[:].rearrange("b (c f) -> b c f", f=1),
                                     mask16[:].rearrange("b (c f) -> b c f", f=1),
                                     allst[:, :, 2:3],
                                     bigc[:].rearrange("b (c f) -> b c f", f=1))
                    tokf = sp.tile([128, 1], F32, tag="tokf", name=f"tokf{t}")
                    nc.vector.tensor_reduce(tokf[:], c16[:],
                                            axis=mybir.AxisListType.X, op=ALU.min)
                    tok_u = sp.tile([128, 1], U32, tag="toku", name=f"toku{t}")
                    nc.vector.tensor_copy(tok_u[:], tokf[:])
                    # gather the next input embedding rows
                    x_sb = sp.tile([B, H], F32, tag="xsb", bufs=1, name=f"xsb{t}")
                    nc.gpsimd.indirect_dma_start(
                        x_sb[:], None, emb_t[:, :],
                        IndirectOffsetOnAxis(ap=tok_u[0:B, :], axis=0))

                prev = (lg, lse_bc)

            # ---- final step write-back + hidden state -----------------
            p_lg, p_lse = prev
            outb = sp.tile([128, HALF], F32, tag="outb", bufs=1, name="outbF")
            nc.gpsimd.tensor_scalar(outb[:], p_lg[:], p_lse[:], None,
                                    op0=ALU.subtract)
            nc.sync.dma_start(out_logp[:, T - 1:T, 0:HALF],
                              outb[0:B, :].rearrange("b (x v) -> b x v", x=1))
            nc.sync.dma_start(out_logp[:, T - 1:T, HALF:VS],
                              outb[B:128, :].rearrange("b (x v) -> b x v", x=1))
            nc.sync.dma_start(out_hsl[:, :], h_sl[:])

    nc.compile()
    return nc


def _prep_core_inputs(c, encoder_hidden, emb, W_ih, W_hh, b_ih, b_hh, W_out, b_out):
    HSl = slice(c * HS, (c + 1) * HS)
    gate_rows = np.r_[c * HS:(c + 1) * HS,
                      H + c * HS:H + (c + 1) * HS,
                      2 * H + c * HS:2 * H + (c + 1) * HS]
    rz_rows = gate_rows[:2 * HS]
    n_rows = gate_rows[2 * HS:]

    def to_kpb(a):  # [rows, H] weight slice -> [128, KC, rows] stationary layout
        return np.ascontiguousarray(
            a.T.reshape(KC, 128, a.shape[0]).transpose(1, 0, 2))

    h0 = encoder_hidden[0]                      # [B, H]
    x0 = np.broadcast_to(emb[SOS], (B, H))      # [B, H]
    return {
        "emb": np.ascontiguousarray(emb),
        "wot": to_kpb(W_out[c * VS:(c + 1) * VS]),
        "wih": to_kpb(W_ih[gate_rows]),
        "whh": to_kpb(W_hh[gate_rows]),
        "bo": b_out[c * VS:(c + 1) * VS][None, :],
        "brz": (b_ih[rz_rows] + b_hh[rz_rows])[None, :],
        "bin": b_ih[n_rows][None, :],
        "bhn": b_hh[n_rows][None, :],
        "h0sl": np.ascontiguousarray(h0[:, HSl]),
        "h0T": np.ascontiguousarray(h0.T.reshape(KC, 128, B).transpose(1, 0, 2)),
        "x0T": np.ascontiguousarray(x0.T.reshape(KC, 128, B).transpose(1, 0, 2)),
        "ident": np.eye(128, dtype=np.float32),
        "voff": np.full((128, 1), c * VS, dtype=np.float32),
    }


def kernel(encoder_outputs, encoder_hidden, emb, W_ih, W_hh, b_ih, b_hh,
           W_out, b_out, _trace=False):
    del encoder_outputs  # unused by the reference decoder (no attention)
    args = [np.asarray(a, dtype=np.float32) for a in
            (encoder_hidden, emb, W_ih, W_hh, b_ih, b_hh, W_out, b_out)]

    if "nc" not in _CACHE:
        _CACHE["nc"] = _build()
    nc = _CACHE["nc"]

    in_maps = [_prep_core_inputs(c, *args) for c in range(NCORE)]
    res = bass_utils.run_bass_kernel_spmd(
        nc, in_maps, core_ids=list(range(NCORE)), trace=_trace)

    log_probs = np.concatenate(
        [res.results[c]["out_logp"] for c in range(NCORE)], axis=2)
    log_probs = np.ascontiguousarray(log_probs.transpose(0, 1, 2))  # [B, T, VS*8]
    hidden = np.concatenate(
        [res.results[c]["out_hsl"] for c in range(NCORE)], axis=1)[None]
    if _trace:
        _CACHE["last_result"] = res
    return log_probs, hidden
